# revision 13
# baseline (speedup 1.0000x reference)
"""GCN layer (PyG GCNConv + ReLU) on 8 Trainium2 NeuronCores.

emb = D^-1/2 (A+I) D^-1/2 (x @ W) + b ; returns (emb, relu(emb))

Strategy v2 (aggregate-then-transform, dst-sharded, host-built one-hots):
  emb = (A_norm @ x) @ W + b  -- algebraically identical, 8x less matmul work
  - Nodes (dst) sharded across 8 cores: core c owns rows [c*12544, (c+1)*12544).
  - x pre-scaled by dinv[src] on host (bf16), replicated in each core's HBM;
    per-edge rows fetched with dma_gather (256 B/row), bucketed by
    (dst block, src chunk) as in v1.
  - Scatter-add realized as PE matmul against HOST-PRECOMPUTED one-hot tiles
    OH[e, d] = w_e * (dstl_e == d) (bf16), streamed contiguously via HWDGE.
    This removes all per-tile DVE/ACT one-hot construction (the v1
    bottleneck: DVE 97% busy).
  - dinv[dst] is folded into the per-partition scale of the output ACT
    copy/relu; the bias seed uses lhsT=1/dinv so the scale leaves b intact.
  - Self-loop tiles are exactly identity (dinv^2 folds into pre/post scale),
    so one constant identity tile serves all blocks.
  - Stage 2 per block: emb[d, fo] = aggT^T @ W(bf16) + b; outputs in bf16,
    host upcasts to f32.

Host does the O(E) graph prep with numpy: degree/norm, (block, chunk)
bucketing, dense one-hot build, uniform padding so one SPMD program serves
all 8 cores.
"""

import numpy as np
import ml_dtypes

import concourse.bass as bass
import concourse.tile as tile
from concourse import bacc, mybir
from concourse.bass_utils import run_bass_kernel_spmd

P = 128            # partitions / tile edge
F = 128            # feature dim (in == out)
NC = 8             # cores
N = 100000         # nodes (full problem)
BLOCKS_PER_CORE = 98
NCHUNK = 4         # src chunks (int16 index range)
GRP = 7            # dst blocks per gather group (must divide BLOCKS_PER_CORE)

BF16 = mybir.dt.bfloat16
F32 = mybir.dt.float32
I16 = mybir.dt.int16

_cache: dict = {}


def _host_prep(x, W, b, edge_index, edge_weight, n_nodes, blocks_per_core,
               n_cores, n_chunks=NCHUNK):
    """Build per-core input maps. Returns (in_maps, Tq)."""
    p = P
    npc = blocks_per_core * p
    n_pad = n_cores * npc
    assert n_pad % n_chunks == 0
    cs = n_pad // n_chunks          # chunk rows
    assert cs < 32768
    n_blocks = n_cores * blocks_per_core

    src = edge_index[0].astype(np.int64)
    dst = edge_index[1].astype(np.int64)
    w = edge_weight.astype(np.float64)

    deg = np.bincount(dst, weights=w, minlength=n_nodes) + 1.0  # + self-loop
    dinv = 1.0 / np.sqrt(deg)                  # [n_nodes]

    blk = dst // p                            # global dst block
    chunk = src // cs                         # src chunk
    seg = blk * n_chunks + chunk              # segment id
    order = np.lexsort((src, seg))            # src-sorted within segment (HBM
    seg_s = seg[order]                        # row locality for the gather)
    n_segs = n_blocks * n_chunks
    cnt = np.bincount(seg_s, minlength=n_segs)
    Tq = max(1, int(np.ceil(cnt.max() / p)))
    Sq = Tq * p                               # padded slots per segment
    til_e = n_chunks * Tq                     # edge tiles per block

    starts = np.zeros(n_segs, dtype=np.int64)
    starts[1:] = np.cumsum(cnt)[:-1]
    pos = np.arange(len(order)) - starts[seg_s]
    slot = seg_s * Sq + pos

    idx_slots = np.zeros(n_segs * Sq, dtype=np.int16)
    idx_slots[slot] = (src[order] - chunk[order] * cs).astype(np.int16)

    # per-slot metadata for on-chip one-hot build (DVE is_equal+mult):
    # dstl[lane, blk*til_e+u] = dst % 128 ; wcol[...] = w_e (0 for pad slots)
    # (dinv[src] folded into x, dinv[dst] folded into output postscale)
    q_of = slot // Sq % n_chunks
    pos_in_seg = slot % Sq
    u_of = q_of * Tq + pos_in_seg // p
    lane = pos_in_seg % p
    tile_id = (slot // (Sq * n_chunks)) * til_e + u_of
    dstl_meta = np.zeros((n_blocks * til_e, p), dtype=np.float32)
    wcol_meta = np.zeros((n_blocks * til_e, p), dtype=np.float32)
    dstl_meta[tile_id, lane] = (dst[order] % p).astype(np.float32)
    wcol_meta[tile_id, lane] = w[order].astype(np.float32)
    dstl3 = dstl_meta.reshape(n_blocks, til_e, p)   # [blk, u, lane]
    wcol3 = wcol_meta.reshape(n_blocks, til_e, p)

    # x prescaled by dinv[src]
    x_pad = np.zeros((n_pad, F), dtype=ml_dtypes.bfloat16)
    x_pad[:n_nodes] = (x.astype(np.float64) * dinv[:, None]).astype(
        ml_dtypes.bfloat16)

    dinv_pad = np.ones(n_pad)
    dinv_pad[:n_nodes] = dinv
    dinv_blk = dinv_pad.reshape(n_blocks, p)  # [blk, lane]

    w_bf = np.ascontiguousarray(W.astype(ml_dtypes.bfloat16))
    b_f32 = np.ascontiguousarray(b.astype(np.float32).reshape(1, F))
    ident = np.ascontiguousarray(np.eye(p, dtype=np.float32)
                                 .astype(ml_dtypes.bfloat16))
    iota = np.ascontiguousarray(np.broadcast_to(
        np.arange(p, dtype=np.float32), (p, p)).astype(ml_dtypes.bfloat16))

    grp = GRP if blocks_per_core % GRP == 0 else 1
    n_grp = blocks_per_core // grp
    cols_pc = grp * Sq // 16                  # idx cols per call
    n_calls = n_grp * n_chunks

    # idx per core: [128, n_calls*cols_pc] int16, call (g,q) at offset
    # (g*n_chunks+q)*cols_pc; within call, edge i=(bi*Sq+s) wrapped [i%16,i//16]
    idx_seg = idx_slots.reshape(n_blocks, n_chunks, Sq)

    in_maps = []
    for c in range(n_cores):
        b0 = c * blocks_per_core
        core_blocks = idx_seg[b0:b0 + blocks_per_core]   # [bpc, n_chunks, Sq]
        cb = core_blocks.reshape(n_grp, grp, n_chunks, Sq)
        # call (g,q): flat [grp*Sq] -> wrapped [16, grp*Sq/16]
        calls = cb.transpose(0, 2, 1, 3).reshape(n_grp, n_chunks, grp * Sq)
        wrapped = calls.reshape(n_grp, n_chunks, grp * Sq // 16, 16)
        wrapped = wrapped.transpose(0, 1, 3, 2).reshape(n_grp * n_chunks * 16,
                                                        cols_pc)
        # -> [16, n_calls*cols_pc]
        idx16 = wrapped.reshape(n_calls, 16, cols_pc).transpose(1, 0, 2)
        idx16 = np.ascontiguousarray(
            np.tile(idx16.reshape(16, n_calls * cols_pc), (8, 1)))

        in_maps.append({
            "x": x_pad,
            "xself": np.ascontiguousarray(x_pad[c * npc:(c + 1) * npc]),
            "w_in": w_bf,
            "b_in": b_f32,
            "ident_in": ident,
            "iota_in": iota,
            "idx_in": idx16,
            "dstl_in": np.ascontiguousarray(
                dstl3[b0:b0 + blocks_per_core].transpose(2, 0, 1)
                .reshape(p, blocks_per_core * til_e)),
            "wcol_in": np.ascontiguousarray(
                wcol3[b0:b0 + blocks_per_core].transpose(2, 0, 1)
                .reshape(p, blocks_per_core * til_e)),
            "dinv_in": np.ascontiguousarray(
                dinv_blk[b0:b0 + blocks_per_core].T.astype(np.float32)),
            "rdinv_in": np.ascontiguousarray(
                (1.0 / dinv_blk[b0:b0 + blocks_per_core])
                .reshape(1, npc).astype(np.float32)),
        })
    return in_maps, Tq


def _build_program(Tq, n_pad, blocks_per_core, n_chunks):
    """Emit the SPMD Tile program. Same program runs on every core."""
    p = P
    npc = blocks_per_core * p
    til_e = n_chunks * Tq
    Sq = Tq * p
    grp = GRP if blocks_per_core % GRP == 0 else 1
    n_grp = blocks_per_core // grp
    cols_pc = grp * Sq // 16
    n_calls = n_grp * n_chunks

    nc = bacc.Bacc("TRN2", target_bir_lowering=False, debug=False,
                   enable_asserts=False, num_devices=NC,
                   num_swdge_queues=4)

    x_d = nc.dram_tensor("x", [n_pad, F], BF16, kind="ExternalInput")
    xself_d = nc.dram_tensor("xself", [npc, F], BF16, kind="ExternalInput")
    w_d = nc.dram_tensor("w_in", [F, F], BF16, kind="ExternalInput")
    b_d = nc.dram_tensor("b_in", [1, F], F32, kind="ExternalInput")
    ident_d = nc.dram_tensor("ident_in", [p, p], BF16, kind="ExternalInput")
    iota_d = nc.dram_tensor("iota_in", [p, p], BF16, kind="ExternalInput")
    idx_d = nc.dram_tensor("idx_in", [p, n_calls * cols_pc], I16,
                           kind="ExternalInput")
    dstl_d = nc.dram_tensor("dstl_in", [p, blocks_per_core * til_e], F32,
                            kind="ExternalInput")
    wcol_d = nc.dram_tensor("wcol_in", [p, blocks_per_core * til_e], F32,
                            kind="ExternalInput")
    dinv_d = nc.dram_tensor("dinv_in", [p, blocks_per_core], F32,
                            kind="ExternalInput")
    rdinv_d = nc.dram_tensor("rdinv_in", [1, npc], F32, kind="ExternalInput")
    emb_d = nc.dram_tensor("emb_out", [npc, F], BF16, kind="ExternalOutput")
    relu_d = nc.dram_tensor("relu_out", [npc, F], BF16, kind="ExternalOutput")

    emb_v = emb_d.ap().rearrange("(B q) f -> q B f", q=p)    # [p, blocks, F]
    relu_v = relu_d.ap().rearrange("(B q) f -> q B f", q=p)
    xself_v = xself_d.ap().rearrange("(B q) f -> q B f", q=p)

    with tile.TileContext(nc) as tc:
        with (
            tc.tile_pool(name="const", bufs=1) as const_pool,
            tc.tile_pool(name="gather", bufs=3) as gpool,
            tc.tile_pool(name="idxbuf", bufs=3) as idxpool,
            tc.tile_pool(name="ohbuf", bufs=8) as ohpool,
            tc.tile_pool(name="aggsb", bufs=3) as aggpool,
            tc.tile_pool(name="outsb", bufs=2) as outpool,
            tc.tile_pool(name="psum_agg", bufs=2, space="PSUM") as ps_agg,
            tc.tile_pool(name="psum_emb", bufs=2, space="PSUM") as ps_emb,
        ):
            w_sb = const_pool.tile([F, F], BF16)
            nc.sync.dma_start(out=w_sb[:], in_=w_d.ap())
            b_sb = const_pool.tile([1, F], F32)
            nc.sync.dma_start(out=b_sb[:], in_=b_d.ap())
            ident_sb = const_pool.tile([p, p], BF16)
            nc.sync.dma_start(out=ident_sb[:], in_=ident_d.ap())
            iota_sb = const_pool.tile([p, p], BF16)
            nc.sync.dma_start(out=iota_sb[:], in_=iota_d.ap())
            dstl_sb = const_pool.tile([p, blocks_per_core * til_e], F32)
            nc.sync.dma_start(out=dstl_sb[:], in_=dstl_d.ap())
            wcol_sb = const_pool.tile([p, blocks_per_core * til_e], F32)
            nc.sync.dma_start(out=wcol_sb[:], in_=wcol_d.ap())
            dinv_sb = const_pool.tile([p, blocks_per_core], F32)
            nc.sync.dma_start(out=dinv_sb[:], in_=dinv_d.ap())
            rdinv_sb = const_pool.tile([1, npc], F32)
            nc.sync.dma_start(out=rdinv_sb[:], in_=rdinv_d.ap())

            for g in range(n_grp):
                idx_g = idxpool.tile([p, n_chunks * cols_pc], I16, tag="idx")
                nc.sync.dma_start(
                    out=idx_g[:],
                    in_=idx_d.ap()[:, g * n_chunks * cols_pc:
                                   (g + 1) * n_chunks * cols_pc])
                gq = []
                for q in range(n_chunks):
                    gt = gpool.tile([p, grp * Sq], BF16, tag=f"g{q}")
                    nc.gpsimd.dma_gather(
                        out_ap=gt[:].rearrange("q (j f) -> q j f", f=F),
                        in_ap=x_d.ap()[q * (n_pad // n_chunks):
                                       (q + 1) * (n_pad // n_chunks), :],
                        idxs_ap=idx_g[:, q * cols_pc:(q + 1) * cols_pc],
                        num_idxs=grp * Sq,
                        num_idxs_reg=grp * Sq,
                        elem_size=F,
                        single_packet=False,
                        queue_num=(g * n_chunks + q) % 4)
                    gq.append(gt)
                gs = gpool.tile([p, grp * F], BF16, tag="gself")
                nc.sync.dma_start(
                    out=gs[:].rearrange("q (B f) -> q B f", f=F),
                    in_=xself_v[:, g * grp:(g + 1) * grp, :])

                emb_st = outpool.tile([p, grp * F], BF16, tag="emb_st")
                relu_st = outpool.tile([p, grp * F], BF16, tag="relu_st")
                for bi in range(grp):
                    blk = g * grp + bi
                    agg_ps = ps_agg.tile([p, p], F32)
                    for u in range(til_e):
                        q, t = divmod(u, Tq)
                        col = blk * til_e + u
                        oh_t = ohpool.tile([p, p], BF16, tag="oh")
                        nc.vector.tensor_scalar(
                            out=oh_t[:], in0=iota_sb[:],
                            scalar1=dstl_sb[:, col:col + 1],
                            scalar2=wcol_sb[:, col:col + 1],
                            op0=mybir.AluOpType.is_equal,
                            op1=mybir.AluOpType.mult)
                        nc.tensor.matmul(
                            out=agg_ps[:],
                            lhsT=gq[q][:, (bi * Tq + t) * F:
                                       (bi * Tq + t + 1) * F],
                            rhs=oh_t[:],
                            start=(u == 0), stop=False)
                    nc.tensor.matmul(
                        out=agg_ps[:],
                        lhsT=gs[:, bi * F:(bi + 1) * F],
                        rhs=ident_sb[:],
                        start=False, stop=True)

                    agg_sb = aggpool.tile([p, p], BF16)
                    nc.scalar.activation(
                        out=agg_sb[:], in_=agg_ps[:],
                        func=mybir.ActivationFunctionType.Copy)
                    emb_ps = ps_emb.tile([p, F], F32)
                    nc.tensor.matmul(out=emb_ps[:],
                                     lhsT=rdinv_sb[:, blk * p:(blk + 1) * p],
                                     rhs=b_sb[:], start=True, stop=False)
                    nc.tensor.matmul(out=emb_ps[:], lhsT=agg_sb[:],
                                     rhs=w_sb[:], start=False, stop=True)
                    nc.scalar.activation(
                        out=emb_st[:, bi * F:(bi + 1) * F], in_=emb_ps[:],
                        func=mybir.ActivationFunctionType.Copy,
                        scale=dinv_sb[:, blk:blk + 1])
                    nc.scalar.activation(
                        out=relu_st[:, bi * F:(bi + 1) * F], in_=emb_ps[:],
                        func=mybir.ActivationFunctionType.Relu,
                        scale=dinv_sb[:, blk:blk + 1])
                nc.sync.dma_start(
                    out=emb_v[:, g * grp:(g + 1) * grp, :],
                    in_=emb_st[:].rearrange("q (B f) -> q B f", f=F))
                nc.sync.dma_start(
                    out=relu_v[:, g * grp:(g + 1) * grp, :],
                    in_=relu_st[:].rearrange("q (B f) -> q B f", f=F))

    nc.compile()
    return nc


def _get_program(Tq, n_pad, blocks_per_core, n_chunks):
    key = (Tq, n_pad, blocks_per_core, n_chunks)
    if key not in _cache:
        _cache[key] = _build_program(Tq, n_pad, blocks_per_core, n_chunks)
    return _cache[key]


def run(x, W, b, edge_index, edge_weight, n_nodes, blocks_per_core, n_cores,
        n_chunks=NCHUNK, trace=False):
    in_maps, Tq = _host_prep(x, W, b, edge_index, edge_weight,
                             n_nodes, blocks_per_core, n_cores, n_chunks)
    n_pad = n_cores * blocks_per_core * P
    nc = _get_program(Tq, n_pad, blocks_per_core, n_chunks)
    res = run_bass_kernel_spmd(nc, in_maps, list(range(n_cores)), trace=trace)
    emb = np.concatenate([res.results[c]["emb_out"] for c in range(n_cores)],
                         axis=0)[:n_nodes]
    relu = np.concatenate([res.results[c]["relu_out"] for c in range(n_cores)],
                          axis=0)[:n_nodes]
    return (emb.astype(np.float32), relu.astype(np.float32)), res


def kernel(x, W, b, level, edge_index, edge_weight):
    x = np.asarray(x)
    W = np.asarray(W)
    b = np.asarray(b)
    edge_index = np.asarray(edge_index)
    edge_weight = np.asarray(edge_weight)
    (emb, relu), _ = run(x, W, b, edge_index, edge_weight,
                         N, BLOCKS_PER_CORE, NC)
    return emb, relu


# revision 20
# speedup vs baseline: 1.7848x; 1.7848x over previous
"""GCN layer (PyG GCNConv + ReLU) on 8 Trainium2 NeuronCores.

emb = D^-1/2 (A+I) D^-1/2 (x @ W) + b ; returns (emb, relu(emb))

Strategy v2 (aggregate-then-transform, dst-sharded, host-built one-hots):
  emb = (A_norm @ x) @ W + b  -- algebraically identical, 8x less matmul work
  - Nodes (dst) sharded across 8 cores: core c owns rows [c*12544, (c+1)*12544).
  - x pre-scaled by dinv[src] on host (bf16), replicated in each core's HBM;
    per-edge rows fetched with dma_gather (256 B/row), bucketed by
    (dst block, src chunk) as in v1.
  - Scatter-add realized as PE matmul against HOST-PRECOMPUTED one-hot tiles
    OH[e, d] = w_e * (dstl_e == d) (bf16), streamed contiguously via HWDGE.
    This removes all per-tile DVE/ACT one-hot construction (the v1
    bottleneck: DVE 97% busy).
  - dinv[dst] is folded into the per-partition scale of the output ACT
    copy/relu; the bias seed uses lhsT=1/dinv so the scale leaves b intact.
  - Self-loop tiles are exactly identity (dinv^2 folds into pre/post scale),
    so one constant identity tile serves all blocks.
  - Stage 2 per block: emb[d, fo] = aggT^T @ W(bf16) + b; outputs in bf16,
    host upcasts to f32.

Host does the O(E) graph prep with numpy: degree/norm, (block, chunk)
bucketing, dense one-hot build, uniform padding so one SPMD program serves
all 8 cores.
"""

import numpy as np
import ml_dtypes

import concourse.bass as bass
import concourse.tile as tile
from concourse import bacc, mybir
from concourse.bass_utils import run_bass_kernel_spmd

P = 128            # partitions / tile edge
F = 128            # feature dim (in == out)
NC = 8             # cores
N = 100000         # nodes (full problem)
BLOCKS_PER_CORE = 98
NCHUNK = 4         # src chunks (int16 index range)
GRP = 7            # dst blocks per gather group (must divide BLOCKS_PER_CORE)

BF16 = mybir.dt.bfloat16
F32 = mybir.dt.float32
I16 = mybir.dt.int16

_cache: dict = {}


def _host_prep(x, W, b, edge_index, edge_weight, n_nodes, blocks_per_core,
               n_cores, n_chunks=NCHUNK):
    """Build per-core input maps. Returns (in_maps, Tq)."""
    p = P
    npc = blocks_per_core * p
    n_pad = n_cores * npc
    assert n_pad % n_chunks == 0
    cs = n_pad // n_chunks          # chunk rows
    assert cs < 32768
    n_blocks = n_cores * blocks_per_core

    src = edge_index[0].astype(np.int64)
    dst = edge_index[1].astype(np.int64)
    w = edge_weight.astype(np.float64)

    deg = np.bincount(dst, weights=w, minlength=n_nodes) + 1.0  # + self-loop
    dinv = 1.0 / np.sqrt(deg)                  # [n_nodes]

    blk = dst // p                            # global dst block
    chunk = src // cs                         # src chunk
    seg = blk * n_chunks + chunk              # segment id
    order = np.lexsort((src, seg))            # src-sorted within segment (HBM
    seg_s = seg[order]                        # row locality for the gather)
    n_segs = n_blocks * n_chunks
    cnt = np.bincount(seg_s, minlength=n_segs)
    Tq = max(1, int(np.ceil(cnt.max() / p)))
    Sq = Tq * p                               # padded slots per segment
    til_e = n_chunks * Tq                     # edge tiles per block

    starts = np.zeros(n_segs, dtype=np.int64)
    starts[1:] = np.cumsum(cnt)[:-1]
    pos = np.arange(len(order)) - starts[seg_s]
    slot = seg_s * Sq + pos

    idx_slots = np.zeros(n_segs * Sq, dtype=np.int16)
    idx_slots[slot] = (src[order] - chunk[order] * cs).astype(np.int16)

    # dense one-hot build: row = blk*til_e*p + u*p + lane ; col = dst % p
    # value = w_e (dinv[src] folded into x, dinv[dst] folded into postscale)
    q_of = slot // Sq % n_chunks
    pos_in_seg = slot % Sq
    u_of = q_of * Tq + pos_in_seg // p
    lane = pos_in_seg % p
    rows = (slot // (Sq * n_chunks)) * (til_e * p) + u_of * p + lane
    oh = np.zeros((n_blocks * til_e * p, p), dtype=ml_dtypes.bfloat16)
    oh[rows, (dst[order] % p)] = w[order].astype(ml_dtypes.bfloat16)
    oh4 = oh.reshape(n_blocks, til_e, p, p)   # [blk, u, lane, d]

    # x prescaled by dinv[src]
    x_pad = np.zeros((n_pad, F), dtype=ml_dtypes.bfloat16)
    x_pad[:n_nodes] = (x.astype(np.float64) * dinv[:, None]).astype(
        ml_dtypes.bfloat16)

    dinv_pad = np.ones(n_pad)
    dinv_pad[:n_nodes] = dinv
    dinv_blk = dinv_pad.reshape(n_blocks, p)  # [blk, lane]

    w_bf = np.ascontiguousarray(W.astype(ml_dtypes.bfloat16))
    b_f32 = np.ascontiguousarray(b.astype(np.float32).reshape(1, F))
    ident = np.ascontiguousarray(np.eye(p, dtype=np.float32)
                                 .astype(ml_dtypes.bfloat16))

    grp = GRP if blocks_per_core % GRP == 0 else 1
    n_grp = blocks_per_core // grp
    cols_pc = grp * Sq // 16                  # idx cols per call
    n_calls = n_grp * n_chunks

    # idx per core: [128, n_calls*cols_pc] int16, call (g,q) at offset
    # (g*n_chunks+q)*cols_pc; within call, edge i=(bi*Sq+s) wrapped [i%16,i//16]
    idx_seg = idx_slots.reshape(n_blocks, n_chunks, Sq)

    in_maps = []
    for c in range(n_cores):
        b0 = c * blocks_per_core
        core_blocks = idx_seg[b0:b0 + blocks_per_core]   # [bpc, n_chunks, Sq]
        cb = core_blocks.reshape(n_grp, grp, n_chunks, Sq)
        # call (g,q): flat [grp*Sq] -> wrapped [16, grp*Sq/16]
        calls = cb.transpose(0, 2, 1, 3).reshape(n_grp, n_chunks, grp * Sq)
        wrapped = calls.reshape(n_grp, n_chunks, grp * Sq // 16, 16)
        wrapped = wrapped.transpose(0, 1, 3, 2).reshape(n_grp * n_chunks * 16,
                                                        cols_pc)
        # -> [16, n_calls*cols_pc]
        idx16 = wrapped.reshape(n_calls, 16, cols_pc).transpose(1, 0, 2)
        idx16 = np.ascontiguousarray(
            np.tile(idx16.reshape(16, n_calls * cols_pc), (8, 1)))

        # one-hots: [lane, blk_local, u, d] -> [128, bpc*til_e*128]
        oh_core = np.ascontiguousarray(
            oh4[b0:b0 + blocks_per_core].transpose(2, 0, 1, 3)
            .reshape(p, blocks_per_core * til_e * p))

        in_maps.append({
            "x": x_pad,
            "xself": np.ascontiguousarray(x_pad[c * npc:(c + 1) * npc]),
            "w_in": w_bf,
            "b_in": b_f32,
            "ident_in": ident,
            "idx_in": idx16,
            "oh_in": oh_core,
            "dinv_in": np.ascontiguousarray(
                dinv_blk[b0:b0 + blocks_per_core].T.astype(np.float32)),
            "rdinv_in": np.ascontiguousarray(
                (1.0 / dinv_blk[b0:b0 + blocks_per_core])
                .reshape(1, npc).astype(np.float32)),
        })
    return in_maps, Tq


def _build_program(Tq, n_pad, blocks_per_core, n_chunks):
    """Emit the SPMD Tile program. Same program runs on every core."""
    p = P
    npc = blocks_per_core * p
    til_e = n_chunks * Tq
    Sq = Tq * p
    grp = GRP if blocks_per_core % GRP == 0 else 1
    n_grp = blocks_per_core // grp
    cols_pc = grp * Sq // 16
    n_calls = n_grp * n_chunks

    nc = bacc.Bacc("TRN2", target_bir_lowering=False, debug=False,
                   enable_asserts=False, num_devices=NC,
                   num_swdge_queues=4)

    x_d = nc.dram_tensor("x", [n_pad, F], BF16, kind="ExternalInput")
    xself_d = nc.dram_tensor("xself", [npc, F], BF16, kind="ExternalInput")
    w_d = nc.dram_tensor("w_in", [F, F], BF16, kind="ExternalInput")
    b_d = nc.dram_tensor("b_in", [1, F], F32, kind="ExternalInput")
    ident_d = nc.dram_tensor("ident_in", [p, p], BF16, kind="ExternalInput")
    idx_d = nc.dram_tensor("idx_in", [p, n_calls * cols_pc], I16,
                           kind="ExternalInput")
    oh_d = nc.dram_tensor("oh_in", [p, blocks_per_core * til_e * p], BF16,
                          kind="ExternalInput")
    dinv_d = nc.dram_tensor("dinv_in", [p, blocks_per_core], F32,
                            kind="ExternalInput")
    rdinv_d = nc.dram_tensor("rdinv_in", [1, npc], F32, kind="ExternalInput")
    emb_d = nc.dram_tensor("emb_out", [npc, F], BF16, kind="ExternalOutput")
    relu_d = nc.dram_tensor("relu_out", [npc, F], BF16, kind="ExternalOutput")

    emb_v = emb_d.ap().rearrange("(B q) f -> q B f", q=p)    # [p, blocks, F]
    relu_v = relu_d.ap().rearrange("(B q) f -> q B f", q=p)
    xself_v = xself_d.ap().rearrange("(B q) f -> q B f", q=p)

    with tile.TileContext(nc) as tc:
        with (
            tc.tile_pool(name="const", bufs=1) as const_pool,
            tc.tile_pool(name="gather", bufs=2) as gpool,
            tc.tile_pool(name="idxbuf", bufs=2) as idxpool,
            tc.tile_pool(name="ohbuf", bufs=2) as ohpool,
            tc.tile_pool(name="aggsb", bufs=3) as aggpool,
            tc.tile_pool(name="outsb", bufs=2) as outpool,
            tc.tile_pool(name="psum_agg", bufs=2, space="PSUM") as ps_agg,
            tc.tile_pool(name="psum_emb", bufs=2, space="PSUM") as ps_emb,
        ):
            w_sb = const_pool.tile([F, F], BF16)
            nc.sync.dma_start(out=w_sb[:], in_=w_d.ap())
            b_sb = const_pool.tile([1, F], F32)
            nc.sync.dma_start(out=b_sb[:], in_=b_d.ap())
            ident_sb = const_pool.tile([p, p], BF16)
            nc.sync.dma_start(out=ident_sb[:], in_=ident_d.ap())
            dinv_sb = const_pool.tile([p, blocks_per_core], F32)
            nc.sync.dma_start(out=dinv_sb[:], in_=dinv_d.ap())
            rdinv_sb = const_pool.tile([1, npc], F32)
            nc.sync.dma_start(out=rdinv_sb[:], in_=rdinv_d.ap())

            oh_v = oh_d.ap()

            for g in range(n_grp):
                idx_g = idxpool.tile([p, n_chunks * cols_pc], I16, tag="idx")
                nc.sync.dma_start(
                    out=idx_g[:],
                    in_=idx_d.ap()[:, g * n_chunks * cols_pc:
                                   (g + 1) * n_chunks * cols_pc])
                gq = []
                for q in range(n_chunks):
                    gt = gpool.tile([p, grp * Sq], BF16, tag=f"g{q}")
                    nc.gpsimd.dma_gather(
                        out_ap=gt[:].rearrange("q (j f) -> q j f", f=F),
                        in_ap=x_d.ap()[q * (n_pad // n_chunks):
                                       (q + 1) * (n_pad // n_chunks), :],
                        idxs_ap=idx_g[:, q * cols_pc:(q + 1) * cols_pc],
                        num_idxs=grp * Sq,
                        num_idxs_reg=grp * Sq,
                        elem_size=F,
                        single_packet=False,
                        queue_num=(g * n_chunks + q) % 4)
                    gq.append(gt)
                gs = gpool.tile([p, grp * F], BF16, tag="gself")
                nc.sync.dma_start(
                    out=gs[:].rearrange("q (B f) -> q B f", f=F),
                    in_=xself_v[:, g * grp:(g + 1) * grp, :])
                oh_sb = ohpool.tile([p, grp * til_e * p], BF16, tag="oh")
                nc.sync.dma_start(
                    out=oh_sb[:],
                    in_=oh_v[:, g * grp * til_e * p:(g + 1) * grp * til_e * p])

                emb_st = outpool.tile([p, grp * F], BF16, tag="emb_st")
                relu_st = outpool.tile([p, grp * F], BF16, tag="relu_st")
                for bi in range(grp):
                    blk = g * grp + bi
                    agg_ps = ps_agg.tile([p, p], F32)
                    for u in range(til_e):
                        q, t = divmod(u, Tq)
                        nc.tensor.matmul(
                            out=agg_ps[:],
                            lhsT=gq[q][:, (bi * Tq + t) * F:
                                       (bi * Tq + t + 1) * F],
                            rhs=oh_sb[:, (bi * til_e + u) * p:
                                      (bi * til_e + u + 1) * p],
                            start=(u == 0), stop=False)
                    nc.tensor.matmul(
                        out=agg_ps[:],
                        lhsT=gs[:, bi * F:(bi + 1) * F],
                        rhs=ident_sb[:],
                        start=False, stop=True)

                    agg_sb = aggpool.tile([p, p], BF16)
                    nc.scalar.activation(
                        out=agg_sb[:], in_=agg_ps[:],
                        func=mybir.ActivationFunctionType.Copy)
                    emb_ps = ps_emb.tile([p, F], F32)
                    nc.tensor.matmul(out=emb_ps[:],
                                     lhsT=rdinv_sb[:, blk * p:(blk + 1) * p],
                                     rhs=b_sb[:], start=True, stop=False)
                    nc.tensor.matmul(out=emb_ps[:], lhsT=agg_sb[:],
                                     rhs=w_sb[:], start=False, stop=True)
                    nc.scalar.activation(
                        out=emb_st[:, bi * F:(bi + 1) * F], in_=emb_ps[:],
                        func=mybir.ActivationFunctionType.Copy,
                        scale=dinv_sb[:, blk:blk + 1])
                    nc.scalar.activation(
                        out=relu_st[:, bi * F:(bi + 1) * F], in_=emb_ps[:],
                        func=mybir.ActivationFunctionType.Relu,
                        scale=dinv_sb[:, blk:blk + 1])
                nc.sync.dma_start(
                    out=emb_v[:, g * grp:(g + 1) * grp, :],
                    in_=emb_st[:].rearrange("q (B f) -> q B f", f=F))
                nc.sync.dma_start(
                    out=relu_v[:, g * grp:(g + 1) * grp, :],
                    in_=relu_st[:].rearrange("q (B f) -> q B f", f=F))

    nc.compile()
    return nc


def _get_program(Tq, n_pad, blocks_per_core, n_chunks):
    key = (Tq, n_pad, blocks_per_core, n_chunks)
    if key not in _cache:
        _cache[key] = _build_program(Tq, n_pad, blocks_per_core, n_chunks)
    return _cache[key]


def run(x, W, b, edge_index, edge_weight, n_nodes, blocks_per_core, n_cores,
        n_chunks=NCHUNK, trace=False):
    in_maps, Tq = _host_prep(x, W, b, edge_index, edge_weight,
                             n_nodes, blocks_per_core, n_cores, n_chunks)
    n_pad = n_cores * blocks_per_core * P
    nc = _get_program(Tq, n_pad, blocks_per_core, n_chunks)
    res = run_bass_kernel_spmd(nc, in_maps, list(range(n_cores)), trace=trace)
    emb = np.concatenate([res.results[c]["emb_out"] for c in range(n_cores)],
                         axis=0)[:n_nodes]
    relu = np.concatenate([res.results[c]["relu_out"] for c in range(n_cores)],
                          axis=0)[:n_nodes]
    return (emb.astype(np.float32), relu.astype(np.float32)), res


def kernel(x, W, b, level, edge_index, edge_weight):
    x = np.asarray(x)
    W = np.asarray(W)
    b = np.asarray(b)
    edge_index = np.asarray(edge_index)
    edge_weight = np.asarray(edge_weight)
    (emb, relu), _ = run(x, W, b, edge_index, edge_weight,
                         N, BLOCKS_PER_CORE, NC)
    return emb, relu
